# revision 10
# baseline (speedup 1.0000x reference)
"""Trainium2 Bass kernel for nn_AttentionBlock (GroupNorm -> QKV 1x1 -> spatial
self-attention -> out-proj + residual), sharded over 8 NeuronCores.

Sharding: data-parallel over batch (2) x query-block (4). Each core gets its
batch image with pixel columns rolled so its 1024 queries are columns 0:1024
(attention + GroupNorm are permutation-invariant over key pixels), computes
K/V over all 4096 keys, and emits its (512, 1024) output slice.

Numerics: all matmuls in bf16 with f32 PSUM accumulation; GroupNorm stats,
softmax normalization and residual in f32.  bk is dropped (additive per-query
score constant, softmax-invariant); bv is folded into the output-proj bias
(softmax rows sum to 1), so bo_eff = bo + wo @ bv.
"""

import numpy as np
import ml_dtypes

import concourse.bass as bass
import concourse.bacc as bacc
import concourse.mybir as mybir
import concourse.tile as tile

F32 = mybir.dt.float32
BF16 = mybir.dt.bfloat16
AF = mybir.ActivationFunctionType
ALU = mybir.AluOpType

P = 128
C = 512          # channels
CT = C // P      # 4 channel tiles
NK = 4096        # key pixels per batch image
KT = NK // P     # 32 key tiles
NQ = 1024        # queries per core
FD = 512         # matmul free-dim chunk
NCH = NK // FD   # 8 column chunks
G = 32           # groups
GS = C // G      # 16 channels per group
EPS = 1e-5
SCALE = float(C) ** -0.5
N_CORES = 8


def build_bass():
    nc = bacc.Bacc("TRN2", target_bir_lowering=False, debug=False,
                   num_devices=N_CORES)

    x_d = nc.dram_tensor("x", (C, NK), F32, kind="ExternalInput").ap()
    wq_d = nc.dram_tensor("wqT", (CT, P, C), BF16, kind="ExternalInput").ap()
    wk_d = nc.dram_tensor("wkT", (CT, P, C), BF16, kind="ExternalInput").ap()
    wv_d = nc.dram_tensor("wvT", (CT, P, C), BF16, kind="ExternalInput").ap()
    wo_d = nc.dram_tensor("woT", (CT, P, C), BF16, kind="ExternalInput").ap()
    bqs_d = nc.dram_tensor("bqs", (P, CT), F32, kind="ExternalInput").ap()
    boe_d = nc.dram_tensor("boe", (P, CT), F32, kind="ExternalInput").ap()
    gam_d = nc.dram_tensor("gam", (P, CT), F32, kind="ExternalInput").ap()
    bet_d = nc.dram_tensor("bet", (P, CT), F32, kind="ExternalInput").ap()
    indf_d = nc.dram_tensor("indf", (P, CT, G), F32, kind="ExternalInput").ap()
    indb_d = nc.dram_tensor("indb", (P, CT, P), F32, kind="ExternalInput").ap()
    out_d = nc.dram_tensor("out", (C, NQ), F32, kind="ExternalOutput").ap()

    with tile.TileContext(nc) as tc:
        with (
            tc.tile_pool(name="px", bufs=1) as px,
            tc.tile_pool(name="pw", bufs=1) as pw,
            tc.tile_pool(name="pc", bufs=1) as pcst,
            tc.tile_pool(name="ph", bufs=3) as ph,
            tc.tile_pool(name="pkvq", bufs=1) as pkvq,
            tc.tile_pool(name="pe", bufs=3) as pe,
            tc.tile_pool(name="psm", bufs=2) as psm,
            tc.tile_pool(name="po", bufs=2) as po,
            tc.tile_pool(name="ps_u", bufs=4, space="PSUM") as ps_u,
            tc.tile_pool(name="ps_r", bufs=1, space="PSUM") as ps_r,
            tc.tile_pool(name="ps_s", bufs=2, space="PSUM") as ps_s,
            tc.tile_pool(name="ps_m", bufs=1, space="PSUM") as ps_m,
        ):
            # ---- constant / weight loads ----
            w_sb = {}
            for nm, d in (("wq", wq_d), ("wk", wk_d), ("wv", wv_d), ("wo", wo_d)):
                t = pw.tile([P, CT, C], BF16, tag=nm)
                for ct in range(CT):
                    nc.sync.dma_start(out=t[:, ct, :], in_=d[ct])
                w_sb[nm] = t
            bqs_sb = pcst.tile([P, CT], F32, tag="bqs")
            nc.sync.dma_start(out=bqs_sb, in_=bqs_d)
            boe_sb = pcst.tile([P, CT], F32, tag="boe")
            nc.sync.dma_start(out=boe_sb, in_=boe_d)
            gam_sb = pcst.tile([P, CT], F32, tag="gam")
            nc.sync.dma_start(out=gam_sb, in_=gam_d)
            bet_sb = pcst.tile([P, CT], F32, tag="bet")
            nc.sync.dma_start(out=bet_sb, in_=bet_d)
            indf_sb = pcst.tile([P, CT, G], F32, tag="indf")
            nc.sync.dma_start(out=indf_sb, in_=indf_d)
            indb_sb = pcst.tile([P, CT, P], F32, tag="indb")
            nc.sync.dma_start(out=indb_sb, in_=indb_d)
            ones_sb = pcst.tile([P, P], BF16, tag="ones")
            nc.vector.memset(ones_sb, 1.0)
            eps_sb = pcst.tile([P, 1], F32, tag="eps")
            nc.vector.memset(eps_sb, EPS)

            # ---- x load + GroupNorm statistics ----
            x_sb = px.tile([P, CT, NK], F32, tag="x")
            stats = pcst.tile([P, CT, NCH, 6], F32, tag="stats")
            mv = pcst.tile([P, CT, 2], F32, tag="mv")
            cstats = pcst.tile([P, CT, 2], F32, tag="cstats")
            for ct in range(CT):
                nc.sync.dma_start(out=x_sb[:, ct, :], in_=x_d[ct * P:(ct + 1) * P, :])
                for s in range(NCH):
                    nc.vector.bn_stats(out=stats[:, ct, s, :],
                                       in_=x_sb[:, ct, s * FD:(s + 1) * FD])
                nc.vector.bn_aggr(out=mv[:, ct, :], in_=stats[:, ct])
                # cstats = [mean_c, var_c + mean_c^2]
                nc.scalar.activation(out=cstats[:, ct, 1:2], in_=mv[:, ct, 0:1],
                                     func=AF.Square)
                nc.vector.tensor_tensor(cstats[:, ct, 1:2], cstats[:, ct, 1:2],
                                        mv[:, ct, 1:2], ALU.add)
                nc.vector.tensor_copy(out=cstats[:, ct, 0:1], in_=mv[:, ct, 0:1])

            # group combine: [32, 2] = sum_ct indf^T @ cstats  (weights 1/16)
            gps = ps_m.tile([G, 2], F32, tag="m")
            for ct in range(CT):
                nc.tensor.matmul(gps, indf_sb[:, ct, :], cstats[:, ct, :],
                                 start=(ct == 0), stop=(ct == CT - 1))
            gsb = pcst.tile([P, 2], F32, tag="gsb")
            nc.vector.tensor_copy(out=gsb[0:G, :], in_=gps)
            # grhs = [mu_g, rstd_g], zero-padded to 128 partitions
            grhs = pcst.tile([P, 2], F32, tag="grhs")
            nc.vector.memset(grhs, 0.0)
            sq = pcst.tile([P, 1], F32, tag="sq")
            nc.scalar.activation(out=sq[0:G], in_=gsb[0:G, 0:1], func=AF.Square)
            nc.vector.tensor_tensor(sq[0:G], gsb[0:G, 1:2], sq[0:G], ALU.subtract)
            nc.scalar.activation(out=sq[0:G], in_=sq[0:G], func=AF.Sqrt,
                                 bias=eps_sb[0:G])
            nc.vector.tensor_copy(out=grhs[0:G, 0:1], in_=gsb[0:G, 0:1])
            nc.vector.reciprocal(out=grhs[0:G, 1:2], in_=sq[0:G])

            # broadcast to per-channel scale/shift: h = x*A + B
            A_sb = pcst.tile([P, CT], F32, tag="A")
            B_sb = pcst.tile([P, CT], F32, tag="B")
            for ct in range(CT):
                abps = ps_m.tile([P, 2], F32, tag="m")
                nc.tensor.matmul(abps, indb_sb[:, ct, :], grhs, start=True, stop=True)
                nc.vector.tensor_tensor(A_sb[:, ct:ct + 1], abps[:, 1:2],
                                        gam_sb[:, ct:ct + 1], ALU.mult)
                nc.vector.tensor_tensor(B_sb[:, ct:ct + 1], abps[:, 0:1],
                                        A_sb[:, ct:ct + 1], ALU.mult)
                nc.vector.tensor_tensor(B_sb[:, ct:ct + 1], bet_sb[:, ct:ct + 1],
                                        B_sb[:, ct:ct + 1], ALU.subtract)

            # ---- GN apply + Q/K/Vt projections, per 512-column chunk ----
            k_sb = pkvq.tile([P, CT, NK], BF16, tag="K")
            vt_sb = pkvq.tile([P, KT, FD], BF16, tag="Vt")
            q_sb = pkvq.tile([P, CT, NQ], BF16, tag="Q")
            for ch in range(NCH):
                cols = slice(ch * FD, (ch + 1) * FD)
                h_ch = ph.tile([P, CT, FD], BF16, tag="h")
                for ct in range(CT):
                    nc.vector.tensor_scalar(
                        out=h_ch[:, ct, :], in0=x_sb[:, ct, cols],
                        scalar1=A_sb[:, ct:ct + 1], scalar2=B_sb[:, ct:ct + 1],
                        op0=ALU.mult, op1=ALU.add)
                # K chunk: K[ot, cols] = sum_ct wkT[ct][:,ot]^T @ h[ct, cols]
                for ot in range(CT):
                    kps = ps_u.tile([P, FD], F32, tag="u")
                    for ct in range(CT):
                        nc.tensor.matmul(kps,
                                         w_sb["wk"][:, ct, ot * P:(ot + 1) * P],
                                         h_ch[:, ct, :],
                                         start=(ct == 0), stop=(ct == CT - 1))
                    nc.vector.tensor_copy(out=k_sb[:, ot, cols], in_=kps)
                # Vt chunk: Vt[kt] = sum_ct h[ct, kt]^T @ wvT[ct]
                for kk in range(FD // P):
                    kt = ch * (FD // P) + kk
                    vps = ps_u.tile([P, FD], F32, tag="u")
                    for ct in range(CT):
                        nc.tensor.matmul(vps,
                                         h_ch[:, ct, kk * P:(kk + 1) * P],
                                         w_sb["wv"][:, ct, :],
                                         start=(ct == 0), stop=(ct == CT - 1))
                    nc.vector.tensor_copy(out=vt_sb[:, kt, :], in_=vps)
                # Q chunk (first 1024 columns only), scaled by C^-0.5, + bq
                if ch < NQ // FD:
                    for ot in range(CT):
                        qps = ps_u.tile([P, FD], F32, tag="u")
                        for ct in range(CT):
                            nc.tensor.matmul(qps,
                                             w_sb["wq"][:, ct, ot * P:(ot + 1) * P],
                                             h_ch[:, ct, :],
                                             start=(ct == 0), stop=(ct == CT - 1))
                        # SCALE is folded into wqT and bqs host-side
                        nc.scalar.activation(out=q_sb[:, ot, cols], in_=qps,
                                             func=AF.Identity,
                                             bias=bqs_sb[:, ot:ot + 1])

            # ---- attention: St = K^T Q per k-tile, exp, U += Vt^T E, r += 1^T E
            attn_sb = pkvq.tile([P, CT, NQ], BF16, tag="attn")
            for qc in range(NQ // FD):
                qcols = slice(qc * FD, (qc + 1) * FD)
                u_ps = [ps_u.tile([P, FD], F32, tag="u", name=f"u{qc}_{cv}")
                        for cv in range(CT)]
                r_ps = ps_r.tile([P, FD], F32, tag="r")
                prev = None

                def emit_u(e, kt, qc=qc, u_ps=u_ps, r_ps=r_ps):
                    for cv in range(CT):
                        nc.tensor.matmul(u_ps[cv],
                                         vt_sb[:, kt, cv * P:(cv + 1) * P], e,
                                         start=(kt == 0), stop=(kt == KT - 1))
                    nc.tensor.matmul(r_ps, ones_sb, e,
                                     start=(kt == 0), stop=(kt == KT - 1))

                for kt in range(KT):
                    sps = ps_s.tile([P, FD], F32, tag="s")
                    for ct in range(CT):
                        nc.tensor.matmul(sps,
                                         k_sb[:, ct, kt * P:(kt + 1) * P],
                                         q_sb[:, ct, qcols],
                                         start=(ct == 0), stop=(ct == CT - 1))
                    e_sb = pe.tile([P, FD], BF16, tag="e")
                    nc.scalar.activation(out=e_sb, in_=sps, func=AF.Exp)
                    if prev is not None:
                        emit_u(prev[0], prev[1])
                    prev = (e_sb, kt)
                emit_u(prev[0], prev[1])

                rr = psm.tile([P, FD], F32, tag="rr")
                nc.vector.reciprocal(out=rr, in_=r_ps)
                for cv in range(CT):
                    nc.vector.tensor_tensor(attn_sb[:, cv, qcols], u_ps[cv], rr,
                                            ALU.mult)

            # ---- output projection + bias + residual ----
            for ot in range(CT):
                for qc in range(NQ // FD):
                    qcols = slice(qc * FD, (qc + 1) * FD)
                    ops = ps_s.tile([P, FD], F32, tag="s")
                    for cv in range(CT):
                        nc.tensor.matmul(ops,
                                         w_sb["wo"][:, cv, ot * P:(ot + 1) * P],
                                         attn_sb[:, cv, qcols],
                                         start=(cv == 0), stop=(cv == CT - 1))
                    o_sb = po.tile([P, FD], F32, tag="o")
                    nc.vector.tensor_tensor(o_sb, ops, x_sb[:, ot, qcols], ALU.add)
                    nc.vector.tensor_scalar(out=o_sb, in0=o_sb,
                                            scalar1=boe_sb[:, ot:ot + 1],
                                            scalar2=None, op0=ALU.add)
                    nc.sync.dma_start(out=out_d[ot * P:(ot + 1) * P, qcols],
                                      in_=o_sb)
    nc.compile()
    return nc


def make_core_inputs(x, gn_w, gn_b, wq, bq, wk, bk, wv, bv, wo, bo):
    """Build the 8 per-core input maps from full inputs."""
    bf16 = ml_dtypes.bfloat16
    f32 = np.float32
    b = x.shape[0]
    xf = np.ascontiguousarray(np.asarray(x, f32).reshape(b, C, NK))

    def wslice(w):
        wT = np.ascontiguousarray(np.asarray(w, f32).T.astype(bf16))
        return np.ascontiguousarray(wT.reshape(CT, P, C))

    # fold the attention scale into the Q projection weights + bias
    wqT = wslice(np.asarray(wq, f32) * SCALE)
    wkT, wvT, woT = wslice(wk), wslice(wv), wslice(wo)

    def percol(v):  # (512,) -> (128, 4): [p, ct]
        return np.ascontiguousarray(np.asarray(v, f32).reshape(CT, P).T)

    bqs = percol(np.asarray(bq, f32) * SCALE)
    bo_eff = percol(np.asarray(bo, np.float64)
                    + np.asarray(wo, np.float64) @ np.asarray(bv, np.float64))
    gam = percol(gn_w)
    bet = percol(gn_b)

    indf = np.zeros((P, CT, G), f32)
    indb = np.zeros((P, CT, P), f32)
    for ct in range(CT):
        for p in range(P):
            g = (ct * P + p) // GS
            indf[p, ct, g] = 1.0 / GS
            indb[g, ct, p] = 1.0
    shared = dict(wqT=wqT, wkT=wkT, wvT=wvT, woT=woT, bqs=bqs, boe=bo_eff,
                  gam=gam, bet=bet, indf=indf, indb=indb)

    in_maps = []
    for core in range(N_CORES):
        bb, qb = core // 4, core % 4
        qs = qb * NQ
        xr = np.ascontiguousarray(
            np.concatenate([xf[bb][:, qs:], xf[bb][:, :qs]], axis=1))
        in_maps.append(dict(x=xr, **shared))
    return in_maps


_NC_CACHE = None


def _get_nc():
    global _NC_CACHE
    if _NC_CACHE is None:
        _NC_CACHE = build_bass()
    return _NC_CACHE


def run_on_cores(in_maps, **kw):
    from concourse.bass_utils import run_bass_kernel_spmd
    nc = _get_nc()
    return run_bass_kernel_spmd(nc, in_maps, core_ids=list(range(N_CORES)), **kw)


def kernel(**inputs):
    x = np.asarray(inputs["x"])
    b, c, H, W = x.shape
    in_maps = make_core_inputs(**inputs)
    res = run_on_cores(in_maps)
    out = np.zeros((b, C, NK), np.float32)
    for core in range(N_CORES):
        bb, qb = core // 4, core % 4
        out[bb][:, qb * NQ:(qb + 1) * NQ] = res.results[core]["out"]
    return out.reshape(b, c, H, W)


# revision 12
# speedup vs baseline: 1.0561x; 1.0561x over previous
"""Trainium2 Bass kernel for nn_AttentionBlock (GroupNorm -> QKV 1x1 -> spatial
self-attention -> out-proj + residual), sharded over 8 NeuronCores.

Sharding: data-parallel over batch (2) x query-block (4). Each core gets its
batch image with pixel columns rolled so its 1024 queries are columns 0:1024
(attention + GroupNorm are permutation-invariant over key pixels), computes
K/V over all 4096 keys, and emits its (512, 1024) output slice.

Numerics: all matmuls in bf16 with f32 PSUM accumulation; GroupNorm stats,
softmax normalization and residual in f32.  bk is dropped (additive per-query
score constant, softmax-invariant); bv is folded into the output-proj bias
(softmax rows sum to 1), so bo_eff = bo + wo @ bv.
"""

import numpy as np
import ml_dtypes

import concourse.bass as bass
import concourse.bacc as bacc
import concourse.mybir as mybir
import concourse.tile as tile

F32 = mybir.dt.float32
BF16 = mybir.dt.bfloat16
AF = mybir.ActivationFunctionType
ALU = mybir.AluOpType

P = 128
C = 512          # channels
CT = C // P      # 4 channel tiles
NK = 4096        # key pixels per batch image
KT = NK // P     # 32 key tiles
NQ = 1024        # queries per core
FD = 512         # matmul free-dim chunk
NCH = NK // FD   # 8 column chunks
G = 32           # groups
GS = C // G      # 16 channels per group
EPS = 1e-5
SCALE = float(C) ** -0.5
N_CORES = 8


def build_bass():
    nc = bacc.Bacc("TRN2", target_bir_lowering=False, debug=False,
                   num_devices=N_CORES)

    x_d = nc.dram_tensor("x", (C, NK), F32, kind="ExternalInput").ap()
    wq_d = nc.dram_tensor("wqT", (CT, P, C), BF16, kind="ExternalInput").ap()
    wk_d = nc.dram_tensor("wkT", (CT, P, C), BF16, kind="ExternalInput").ap()
    wv_d = nc.dram_tensor("wvT", (CT, P, C), BF16, kind="ExternalInput").ap()
    wo_d = nc.dram_tensor("woT", (CT, P, C), BF16, kind="ExternalInput").ap()
    bqs_d = nc.dram_tensor("bqs", (P, CT), F32, kind="ExternalInput").ap()
    boe_d = nc.dram_tensor("boe", (P, CT), F32, kind="ExternalInput").ap()
    gam_d = nc.dram_tensor("gam", (P, CT), F32, kind="ExternalInput").ap()
    bet_d = nc.dram_tensor("bet", (P, CT), F32, kind="ExternalInput").ap()
    indf_d = nc.dram_tensor("indf", (P, CT, G), F32, kind="ExternalInput").ap()
    indb_d = nc.dram_tensor("indb", (P, CT, P), F32, kind="ExternalInput").ap()
    out_d = nc.dram_tensor("out", (C, NQ), F32, kind="ExternalOutput").ap()

    with tile.TileContext(nc) as tc:
        with (
            tc.tile_pool(name="px", bufs=1) as px,
            tc.tile_pool(name="pw", bufs=1) as pw,
            tc.tile_pool(name="pc", bufs=1) as pcst,
            tc.tile_pool(name="ph", bufs=3) as ph,
            tc.tile_pool(name="pkvq", bufs=1) as pkvq,
            tc.tile_pool(name="pe", bufs=3) as pe,
            tc.tile_pool(name="psm", bufs=2) as psm,
            tc.tile_pool(name="po", bufs=2) as po,
            tc.tile_pool(name="ps_u", bufs=4, space="PSUM") as ps_u,
            tc.tile_pool(name="ps_r", bufs=1, space="PSUM") as ps_r,
            tc.tile_pool(name="ps_s", bufs=2, space="PSUM") as ps_s,
            tc.tile_pool(name="ps_m", bufs=1, space="PSUM") as ps_m,
        ):
            # ---- PE warmup: dummy matmul chain spanning the x-DMA prologue
            # keeps the HAM clock-gate at K=8/8 so the first real matmuls
            # run at 2.4 GHz instead of 1.2 GHz.
            ones_sb = pcst.tile([P, P], BF16, tag="ones")
            nc.vector.memset(ones_sb, 1.0)
            warm_rhs = pcst.tile([P, FD], BF16, tag="wrm")
            nc.vector.memset(warm_rhs, 0.0)
            wsink = pcst.tile([P, 1], F32, tag="wsink")
            NWARM = 140
            wps = ps_m.tile([P, FD], F32, tag="m", name="warm")
            for i in range(NWARM):
                nc.tensor.matmul(wps, ones_sb, warm_rhs,
                                 start=(i == 0), stop=(i == NWARM - 1))
            nc.vector.tensor_copy(out=wsink, in_=wps[:, 0:1])

            # ---- x load (first on the DMA queues) + GroupNorm statistics ----
            x_sb = px.tile([P, CT, NK], F32, tag="x")
            stats = pcst.tile([P, CT, NCH, 6], F32, tag="stats")
            mv = pcst.tile([P, CT, 2], F32, tag="mv")
            cstats = pcst.tile([P, CT, 2], F32, tag="cstats")
            XDH = 2048  # DMA chunk columns (1 MB) so bn_stats trails the DMA
            for ct in range(CT):
                for xc in range(NK // XDH):
                    xcols = slice(xc * XDH, (xc + 1) * XDH)
                    nc.sync.dma_start(out=x_sb[:, ct, xcols],
                                      in_=x_d[ct * P:(ct + 1) * P, xcols])
                    for s in range(xc * (XDH // FD), (xc + 1) * (XDH // FD)):
                        nc.vector.bn_stats(out=stats[:, ct, s, :],
                                           in_=x_sb[:, ct, s * FD:(s + 1) * FD])
                nc.vector.bn_aggr(out=mv[:, ct, :], in_=stats[:, ct])
                # cstats = [mean_c, var_c + mean_c^2]
                nc.scalar.activation(out=cstats[:, ct, 1:2], in_=mv[:, ct, 0:1],
                                     func=AF.Square)
                nc.vector.tensor_tensor(cstats[:, ct, 1:2], cstats[:, ct, 1:2],
                                        mv[:, ct, 1:2], ALU.add)
                nc.vector.tensor_copy(out=cstats[:, ct, 0:1], in_=mv[:, ct, 0:1])

            # ---- constant / weight loads ----
            w_sb = {}
            for nm, d in (("wq", wq_d), ("wk", wk_d), ("wv", wv_d), ("wo", wo_d)):
                t = pw.tile([P, CT, C], BF16, tag=nm)
                for ct in range(CT):
                    nc.sync.dma_start(out=t[:, ct, :], in_=d[ct])
                w_sb[nm] = t
            bqs_sb = pcst.tile([P, CT], F32, tag="bqs")
            nc.sync.dma_start(out=bqs_sb, in_=bqs_d)
            boe_sb = pcst.tile([P, CT], F32, tag="boe")
            nc.sync.dma_start(out=boe_sb, in_=boe_d)
            gam_sb = pcst.tile([P, CT], F32, tag="gam")
            nc.sync.dma_start(out=gam_sb, in_=gam_d)
            bet_sb = pcst.tile([P, CT], F32, tag="bet")
            nc.sync.dma_start(out=bet_sb, in_=bet_d)
            indf_sb = pcst.tile([P, CT, G], F32, tag="indf")
            nc.sync.dma_start(out=indf_sb, in_=indf_d)
            indb_sb = pcst.tile([P, CT, P], F32, tag="indb")
            nc.sync.dma_start(out=indb_sb, in_=indb_d)
            eps_sb = pcst.tile([P, 1], F32, tag="eps")
            nc.vector.memset(eps_sb, EPS)

            # group combine: [32, 2] = sum_ct indf^T @ cstats  (weights 1/16)
            gps = ps_m.tile([G, 2], F32, tag="m")
            for ct in range(CT):
                nc.tensor.matmul(gps, indf_sb[:, ct, :], cstats[:, ct, :],
                                 start=(ct == 0), stop=(ct == CT - 1))
            gsb = pcst.tile([P, 2], F32, tag="gsb")
            nc.vector.tensor_copy(out=gsb[0:G, :], in_=gps)
            # grhs = [mu_g, rstd_g], zero-padded to 128 partitions
            grhs = pcst.tile([P, 2], F32, tag="grhs")
            nc.vector.memset(grhs, 0.0)
            sq = pcst.tile([P, 1], F32, tag="sq")
            nc.scalar.activation(out=sq[0:G], in_=gsb[0:G, 0:1], func=AF.Square)
            nc.vector.tensor_tensor(sq[0:G], gsb[0:G, 1:2], sq[0:G], ALU.subtract)
            nc.scalar.activation(out=sq[0:G], in_=sq[0:G], func=AF.Sqrt,
                                 bias=eps_sb[0:G])
            nc.vector.tensor_copy(out=grhs[0:G, 0:1], in_=gsb[0:G, 0:1])
            nc.vector.reciprocal(out=grhs[0:G, 1:2], in_=sq[0:G])

            # broadcast to per-channel scale/shift: h = x*A + B
            A_sb = pcst.tile([P, CT], F32, tag="A")
            B_sb = pcst.tile([P, CT], F32, tag="B")
            for ct in range(CT):
                abps = ps_m.tile([P, 2], F32, tag="m")
                nc.tensor.matmul(abps, indb_sb[:, ct, :], grhs, start=True, stop=True)
                nc.vector.tensor_tensor(A_sb[:, ct:ct + 1], abps[:, 1:2],
                                        gam_sb[:, ct:ct + 1], ALU.mult)
                nc.vector.tensor_tensor(B_sb[:, ct:ct + 1], abps[:, 0:1],
                                        A_sb[:, ct:ct + 1], ALU.mult)
                nc.vector.tensor_tensor(B_sb[:, ct:ct + 1], bet_sb[:, ct:ct + 1],
                                        B_sb[:, ct:ct + 1], ALU.subtract)

            # ---- GN apply + Q/K/Vt projections, per 512-column chunk ----
            k_sb = pkvq.tile([P, CT, NK], BF16, tag="K")
            vt_sb = pkvq.tile([P, KT, FD], BF16, tag="Vt")
            q_sb = pkvq.tile([P, CT, NQ], BF16, tag="Q")
            for ch in range(NCH):
                cols = slice(ch * FD, (ch + 1) * FD)
                h_ch = ph.tile([P, CT, FD], BF16, tag="h")
                for ct in range(CT):
                    nc.vector.tensor_scalar(
                        out=h_ch[:, ct, :], in0=x_sb[:, ct, cols],
                        scalar1=A_sb[:, ct:ct + 1], scalar2=B_sb[:, ct:ct + 1],
                        op0=ALU.mult, op1=ALU.add)
                # K chunk: K[ot, cols] = sum_ct wkT[ct][:,ot]^T @ h[ct, cols]
                for ot in range(CT):
                    kps = ps_u.tile([P, FD], F32, tag="u")
                    for ct in range(CT):
                        nc.tensor.matmul(kps,
                                         w_sb["wk"][:, ct, ot * P:(ot + 1) * P],
                                         h_ch[:, ct, :],
                                         start=(ct == 0), stop=(ct == CT - 1))
                    nc.vector.tensor_copy(out=k_sb[:, ot, cols], in_=kps)
                # Vt chunk: Vt[kt] = sum_ct h[ct, kt]^T @ wvT[ct]
                for kk in range(FD // P):
                    kt = ch * (FD // P) + kk
                    vps = ps_u.tile([P, FD], F32, tag="u")
                    for ct in range(CT):
                        nc.tensor.matmul(vps,
                                         h_ch[:, ct, kk * P:(kk + 1) * P],
                                         w_sb["wv"][:, ct, :],
                                         start=(ct == 0), stop=(ct == CT - 1))
                    nc.vector.tensor_copy(out=vt_sb[:, kt, :], in_=vps)
                # Q chunk (first 1024 columns only), scaled by C^-0.5, + bq
                if ch < NQ // FD:
                    for ot in range(CT):
                        qps = ps_u.tile([P, FD], F32, tag="u")
                        for ct in range(CT):
                            nc.tensor.matmul(qps,
                                             w_sb["wq"][:, ct, ot * P:(ot + 1) * P],
                                             h_ch[:, ct, :],
                                             start=(ct == 0), stop=(ct == CT - 1))
                        # SCALE is folded into wqT and bqs host-side
                        nc.scalar.activation(out=q_sb[:, ot, cols], in_=qps,
                                             func=AF.Identity,
                                             bias=bqs_sb[:, ot:ot + 1])

            # ---- attention: St = K^T Q per k-tile, exp, U += Vt^T E, r += 1^T E
            attn_sb = pkvq.tile([P, CT, NQ], BF16, tag="attn")
            for qc in range(NQ // FD):
                qcols = slice(qc * FD, (qc + 1) * FD)
                u_ps = [ps_u.tile([P, FD], F32, tag="u", name=f"u{qc}_{cv}")
                        for cv in range(CT)]
                r_ps = ps_r.tile([P, FD], F32, tag="r")
                prev = None

                def emit_u(e, kt, qc=qc, u_ps=u_ps, r_ps=r_ps):
                    for cv in range(CT):
                        nc.tensor.matmul(u_ps[cv],
                                         vt_sb[:, kt, cv * P:(cv + 1) * P], e,
                                         start=(kt == 0), stop=(kt == KT - 1))
                    nc.tensor.matmul(r_ps, ones_sb, e,
                                     start=(kt == 0), stop=(kt == KT - 1))

                for kt in range(KT):
                    sps = ps_s.tile([P, FD], F32, tag="s")
                    for ct in range(CT):
                        nc.tensor.matmul(sps,
                                         k_sb[:, ct, kt * P:(kt + 1) * P],
                                         q_sb[:, ct, qcols],
                                         start=(ct == 0), stop=(ct == CT - 1))
                    e_sb = pe.tile([P, FD], BF16, tag="e")
                    nc.scalar.activation(out=e_sb, in_=sps, func=AF.Exp)
                    if prev is not None:
                        emit_u(prev[0], prev[1])
                    prev = (e_sb, kt)
                emit_u(prev[0], prev[1])

                rr = psm.tile([P, FD], F32, tag="rr")
                nc.vector.reciprocal(out=rr, in_=r_ps)
                for cv in range(CT):
                    nc.vector.tensor_tensor(attn_sb[:, cv, qcols], u_ps[cv], rr,
                                            ALU.mult)

                # output projection + bias + residual for this q-chunk;
                # qc=0's projection overlaps qc=1's attention on the PE.
                proj_pool = ps_m if qc == 0 else ps_s
                for ot in range(CT):
                    ops = proj_pool.tile([P, FD], F32,
                                         tag="m" if qc == 0 else "s",
                                         name=f"proj{qc}_{ot}")
                    for cv in range(CT):
                        nc.tensor.matmul(ops,
                                         w_sb["wo"][:, cv, ot * P:(ot + 1) * P],
                                         attn_sb[:, cv, qcols],
                                         start=(cv == 0), stop=(cv == CT - 1))
                    o_sb = po.tile([P, FD], F32, tag="o", name=f"o{qc}_{ot}")
                    nc.vector.tensor_tensor(o_sb, ops, x_sb[:, ot, qcols], ALU.add)
                    nc.vector.tensor_scalar(out=o_sb, in0=o_sb,
                                            scalar1=boe_sb[:, ot:ot + 1],
                                            scalar2=None, op0=ALU.add)
                    nc.sync.dma_start(out=out_d[ot * P:(ot + 1) * P, qcols],
                                      in_=o_sb)
    nc.compile()
    return nc


def make_core_inputs(x, gn_w, gn_b, wq, bq, wk, bk, wv, bv, wo, bo):
    """Build the 8 per-core input maps from full inputs."""
    bf16 = ml_dtypes.bfloat16
    f32 = np.float32
    b = x.shape[0]
    xf = np.ascontiguousarray(np.asarray(x, f32).reshape(b, C, NK))

    def wslice(w):
        wT = np.ascontiguousarray(np.asarray(w, f32).T.astype(bf16))
        return np.ascontiguousarray(wT.reshape(CT, P, C))

    # fold the attention scale into the Q projection weights + bias
    wqT = wslice(np.asarray(wq, f32) * SCALE)
    wkT, wvT, woT = wslice(wk), wslice(wv), wslice(wo)

    def percol(v):  # (512,) -> (128, 4): [p, ct]
        return np.ascontiguousarray(np.asarray(v, f32).reshape(CT, P).T)

    bqs = percol(np.asarray(bq, f32) * SCALE)
    bo_eff = percol(np.asarray(bo, np.float64)
                    + np.asarray(wo, np.float64) @ np.asarray(bv, np.float64))
    gam = percol(gn_w)
    bet = percol(gn_b)

    indf = np.zeros((P, CT, G), f32)
    indb = np.zeros((P, CT, P), f32)
    for ct in range(CT):
        for p in range(P):
            g = (ct * P + p) // GS
            indf[p, ct, g] = 1.0 / GS
            indb[g, ct, p] = 1.0
    shared = dict(wqT=wqT, wkT=wkT, wvT=wvT, woT=woT, bqs=bqs, boe=bo_eff,
                  gam=gam, bet=bet, indf=indf, indb=indb)

    in_maps = []
    for core in range(N_CORES):
        bb, qb = core // 4, core % 4
        qs = qb * NQ
        xr = np.ascontiguousarray(
            np.concatenate([xf[bb][:, qs:], xf[bb][:, :qs]], axis=1))
        in_maps.append(dict(x=xr, **shared))
    return in_maps


_NC_CACHE = None


def _get_nc():
    global _NC_CACHE
    if _NC_CACHE is None:
        _NC_CACHE = build_bass()
    return _NC_CACHE


def run_on_cores(in_maps, **kw):
    from concourse.bass_utils import run_bass_kernel_spmd
    nc = _get_nc()
    return run_bass_kernel_spmd(nc, in_maps, core_ids=list(range(N_CORES)), **kw)


def kernel(**inputs):
    x = np.asarray(inputs["x"])
    b, c, H, W = x.shape
    in_maps = make_core_inputs(**inputs)
    res = run_on_cores(in_maps)
    out = np.zeros((b, C, NK), np.float32)
    for core in range(N_CORES):
        bb, qb = core // 4, core % 4
        out[bb][:, qb * NQ:(qb + 1) * NQ] = res.results[core]["out"]
    return out.reshape(b, c, H, W)


# revision 14
# speedup vs baseline: 1.1575x; 1.0960x over previous
"""Trainium2 Bass kernel for nn_AttentionBlock (GroupNorm -> QKV 1x1 -> spatial
self-attention -> out-proj + residual), sharded over 8 NeuronCores.

Sharding: data-parallel over batch (2) x query-block (4). Each core gets its
batch image with pixel columns rolled so its 1024 queries are columns 0:1024
(attention + GroupNorm are permutation-invariant over key pixels), computes
K/V over all 4096 keys, and emits its (512, 1024) output slice.

Numerics: all matmuls in bf16 with f32 PSUM accumulation; GroupNorm stats,
softmax normalization and residual in f32.  bk is dropped (additive per-query
score constant, softmax-invariant); bv is folded into the output-proj bias
(softmax rows sum to 1), so bo_eff = bo + wo @ bv.
"""

import numpy as np
import ml_dtypes

import concourse.bass as bass
import concourse.bacc as bacc
import concourse.mybir as mybir
import concourse.tile as tile

F32 = mybir.dt.float32
BF16 = mybir.dt.bfloat16
FP8 = mybir.dt.float8e4
DR = mybir.MatmulPerfMode.DoubleRow
AF = mybir.ActivationFunctionType
ALU = mybir.AluOpType

P = 128
C = 512          # channels
CT = C // P      # 4 channel tiles
NK = 4096        # key pixels per batch image
KT = NK // P     # 32 key tiles
NQ = 1024        # queries per core
FD = 512         # matmul free-dim chunk
NCH = NK // FD   # 8 column chunks
G = 32           # groups
GS = C // G      # 16 channels per group
EPS = 1e-5
SCALE = float(C) ** -0.5
N_CORES = 8


def build_bass():
    nc = bacc.Bacc("TRN2", target_bir_lowering=False, debug=False,
                   num_devices=N_CORES)

    x_d = nc.dram_tensor("x", (C, NK), F32, kind="ExternalInput").ap()
    wq_d = nc.dram_tensor("wqT", (CT, P, C), BF16, kind="ExternalInput").ap()
    wk_d = nc.dram_tensor("wkT", (CT, P, C), BF16, kind="ExternalInput").ap()
    wv_d = nc.dram_tensor("wvT", (CT, P, C), BF16, kind="ExternalInput").ap()
    wo_d = nc.dram_tensor("woT", (CT, P, C), BF16, kind="ExternalInput").ap()
    bqs_d = nc.dram_tensor("bqs", (P, CT), F32, kind="ExternalInput").ap()
    boe_d = nc.dram_tensor("boe", (P, CT), F32, kind="ExternalInput").ap()
    gam_d = nc.dram_tensor("gam", (P, CT), F32, kind="ExternalInput").ap()
    bet_d = nc.dram_tensor("bet", (P, CT), F32, kind="ExternalInput").ap()
    indf_d = nc.dram_tensor("indf", (P, CT, G), F32, kind="ExternalInput").ap()
    indb_d = nc.dram_tensor("indb", (P, CT, P), F32, kind="ExternalInput").ap()
    out_d = nc.dram_tensor("out", (C, NQ), F32, kind="ExternalOutput").ap()

    with tile.TileContext(nc) as tc:
        with (
            tc.tile_pool(name="px", bufs=1) as px,
            tc.tile_pool(name="pw", bufs=1) as pw,
            tc.tile_pool(name="pc", bufs=1) as pcst,
            tc.tile_pool(name="ph", bufs=3) as ph,
            tc.tile_pool(name="pkvq", bufs=1) as pkvq,
            tc.tile_pool(name="pe", bufs=4) as pe,
            tc.tile_pool(name="psm", bufs=2) as psm,
            tc.tile_pool(name="po", bufs=2) as po,
            tc.tile_pool(name="ps_u", bufs=4, space="PSUM") as ps_u,
            tc.tile_pool(name="ps_r", bufs=1, space="PSUM") as ps_r,
            tc.tile_pool(name="ps_s", bufs=2, space="PSUM") as ps_s,
            tc.tile_pool(name="ps_m", bufs=1, space="PSUM") as ps_m,
        ):
            # ---- PE warmup: dummy matmul chain spanning the x-DMA prologue
            # keeps the HAM clock-gate at K=8/8 so the first real matmuls
            # run at 2.4 GHz instead of 1.2 GHz.
            ones_sb = pcst.tile([P, P], BF16, tag="ones")
            nc.vector.memset(ones_sb, 1.0)
            warm_rhs = pcst.tile([P, FD], BF16, tag="wrm")
            nc.vector.memset(warm_rhs, 0.0)
            wsink = pcst.tile([P, 1], F32, tag="wsink")
            NWARM = 140
            wps = ps_m.tile([P, FD], F32, tag="m", name="warm")
            for i in range(NWARM):
                nc.tensor.matmul(wps, ones_sb, warm_rhs,
                                 start=(i == 0), stop=(i == NWARM - 1))
            nc.vector.tensor_copy(out=wsink, in_=wps[:, 0:1])

            # ---- x load (first on the DMA queues) + GroupNorm statistics ----
            x_sb = px.tile([P, CT, NK], F32, tag="x")
            stats = pcst.tile([P, CT, NCH, 6], F32, tag="stats")
            mv = pcst.tile([P, CT, 2], F32, tag="mv")
            cstats = pcst.tile([P, CT, 2], F32, tag="cstats")
            XDH = 2048  # DMA chunk columns (1 MB) so bn_stats trails the DMA
            for ct in range(CT):
                for xc in range(NK // XDH):
                    xcols = slice(xc * XDH, (xc + 1) * XDH)
                    nc.sync.dma_start(out=x_sb[:, ct, xcols],
                                      in_=x_d[ct * P:(ct + 1) * P, xcols])
                    for s in range(xc * (XDH // FD), (xc + 1) * (XDH // FD)):
                        nc.vector.bn_stats(out=stats[:, ct, s, :],
                                           in_=x_sb[:, ct, s * FD:(s + 1) * FD])
                nc.vector.bn_aggr(out=mv[:, ct, :], in_=stats[:, ct])
                # cstats = [mean_c, var_c + mean_c^2]
                nc.scalar.activation(out=cstats[:, ct, 1:2], in_=mv[:, ct, 0:1],
                                     func=AF.Square)
                nc.vector.tensor_tensor(cstats[:, ct, 1:2], cstats[:, ct, 1:2],
                                        mv[:, ct, 1:2], ALU.add)
                nc.vector.tensor_copy(out=cstats[:, ct, 0:1], in_=mv[:, ct, 0:1])

            # ---- constant / weight loads ----
            w_sb = {}
            for nm, d in (("wq", wq_d), ("wk", wk_d), ("wv", wv_d), ("wo", wo_d)):
                t = pw.tile([P, CT, C], BF16, tag=nm)
                for ct in range(CT):
                    nc.sync.dma_start(out=t[:, ct, :], in_=d[ct])
                w_sb[nm] = t
            bqs_sb = pcst.tile([P, CT], F32, tag="bqs")
            nc.sync.dma_start(out=bqs_sb, in_=bqs_d)
            boe_sb = pcst.tile([P, CT], F32, tag="boe")
            nc.sync.dma_start(out=boe_sb, in_=boe_d)
            gam_sb = pcst.tile([P, CT], F32, tag="gam")
            nc.sync.dma_start(out=gam_sb, in_=gam_d)
            bet_sb = pcst.tile([P, CT], F32, tag="bet")
            nc.sync.dma_start(out=bet_sb, in_=bet_d)
            indf_sb = pcst.tile([P, CT, G], F32, tag="indf")
            nc.sync.dma_start(out=indf_sb, in_=indf_d)
            indb_sb = pcst.tile([P, CT, P], F32, tag="indb")
            nc.sync.dma_start(out=indb_sb, in_=indb_d)
            eps_sb = pcst.tile([P, 1], F32, tag="eps")
            nc.vector.memset(eps_sb, EPS)

            # group combine: [32, 2] = sum_ct indf^T @ cstats  (weights 1/16)
            gps = ps_m.tile([G, 2], F32, tag="m")
            for ct in range(CT):
                nc.tensor.matmul(gps, indf_sb[:, ct, :], cstats[:, ct, :],
                                 start=(ct == 0), stop=(ct == CT - 1))
            gsb = pcst.tile([P, 2], F32, tag="gsb")
            nc.vector.tensor_copy(out=gsb[0:G, :], in_=gps)
            # grhs = [mu_g, rstd_g], zero-padded to 128 partitions
            grhs = pcst.tile([P, 2], F32, tag="grhs")
            nc.vector.memset(grhs, 0.0)
            sq = pcst.tile([P, 1], F32, tag="sq")
            nc.scalar.activation(out=sq[0:G], in_=gsb[0:G, 0:1], func=AF.Square)
            nc.vector.tensor_tensor(sq[0:G], gsb[0:G, 1:2], sq[0:G], ALU.subtract)
            nc.scalar.activation(out=sq[0:G], in_=sq[0:G], func=AF.Sqrt,
                                 bias=eps_sb[0:G])
            nc.vector.tensor_copy(out=grhs[0:G, 0:1], in_=gsb[0:G, 0:1])
            nc.vector.reciprocal(out=grhs[0:G, 1:2], in_=sq[0:G])

            # broadcast to per-channel scale/shift: h = x*A + B
            A_sb = pcst.tile([P, CT], F32, tag="A")
            B_sb = pcst.tile([P, CT], F32, tag="B")
            for ct in range(CT):
                abps = ps_m.tile([P, 2], F32, tag="m")
                nc.tensor.matmul(abps, indb_sb[:, ct, :], grhs, start=True, stop=True)
                nc.vector.tensor_tensor(A_sb[:, ct:ct + 1], abps[:, 1:2],
                                        gam_sb[:, ct:ct + 1], ALU.mult)
                nc.vector.tensor_tensor(B_sb[:, ct:ct + 1], abps[:, 0:1],
                                        A_sb[:, ct:ct + 1], ALU.mult)
                nc.vector.tensor_tensor(B_sb[:, ct:ct + 1], bet_sb[:, ct:ct + 1],
                                        B_sb[:, ct:ct + 1], ALU.subtract)

            # ---- GN apply + Q/K/Vt projections, per 512-column chunk ----
            # K/Q/Vt stored as fp8 e4m3 in DoubleRow pair layout [P, pair, 2, n]
            k_sb = pkvq.tile([P, CT // 2, 2, NK], FP8, tag="K")
            vt_sb = pkvq.tile([P, KT // 2, 2, FD], FP8, tag="Vt")
            q_sb = pkvq.tile([P, CT // 2, 2, NQ], FP8, tag="Q")
            onesp_sb = pcst.tile([P, 2, P], FP8, tag="onesp")
            nc.vector.memset(onesp_sb, 1.0)
            for ch in range(NCH):
                cols = slice(ch * FD, (ch + 1) * FD)
                h_ch = ph.tile([P, CT, FD], BF16, tag="h")
                for ct in range(CT):
                    nc.vector.tensor_scalar(
                        out=h_ch[:, ct, :], in0=x_sb[:, ct, cols],
                        scalar1=A_sb[:, ct:ct + 1], scalar2=B_sb[:, ct:ct + 1],
                        op0=ALU.mult, op1=ALU.add)
                # K chunk: K[ot, cols] = sum_ct wkT[ct][:,ot]^T @ h[ct, cols]
                for ot in range(CT):
                    kps = ps_u.tile([P, FD], F32, tag="u")
                    for ct in range(CT):
                        nc.tensor.matmul(kps,
                                         w_sb["wk"][:, ct, ot * P:(ot + 1) * P],
                                         h_ch[:, ct, :],
                                         start=(ct == 0), stop=(ct == CT - 1))
                    nc.vector.tensor_copy(out=k_sb[:, ot // 2, ot % 2, cols],
                                          in_=kps)
                # Vt chunk: Vt[kt] = sum_ct h[ct, kt]^T @ wvT[ct]
                for kk in range(FD // P):
                    kt = ch * (FD // P) + kk
                    vps = ps_u.tile([P, FD], F32, tag="u")
                    for ct in range(CT):
                        nc.tensor.matmul(vps,
                                         h_ch[:, ct, kk * P:(kk + 1) * P],
                                         w_sb["wv"][:, ct, :],
                                         start=(ct == 0), stop=(ct == CT - 1))
                    nc.vector.tensor_copy(out=vt_sb[:, kt // 2, kt % 2, :],
                                          in_=vps)
                # Q chunk (first 1024 columns only), scaled by C^-0.5, + bq
                if ch < NQ // FD:
                    for ot in range(CT):
                        qps = ps_u.tile([P, FD], F32, tag="u")
                        for ct in range(CT):
                            nc.tensor.matmul(qps,
                                             w_sb["wq"][:, ct, ot * P:(ot + 1) * P],
                                             h_ch[:, ct, :],
                                             start=(ct == 0), stop=(ct == CT - 1))
                        # SCALE is folded into wqT and bqs host-side
                        nc.scalar.activation(out=q_sb[:, ot // 2, ot % 2, cols],
                                             in_=qps, func=AF.Identity,
                                             bias=bqs_sb[:, ot:ot + 1])

            # ---- attention: St = K^T Q per k-tile, exp, U += Vt^T E, r += 1^T E
            attn_sb = pkvq.tile([P, CT, NQ], BF16, tag="attn")
            for qc in range(NQ // FD):
                qcols = slice(qc * FD, (qc + 1) * FD)
                u_ps = [ps_u.tile([P, FD], F32, tag="u", name=f"u{qc}_{cv}")
                        for cv in range(CT)]
                r_ps = ps_r.tile([P, FD], F32, tag="r")
                KTP = KT // 2
                pend = []

                def emit_u(ep, ktp, qc=qc, u_ps=u_ps, r_ps=r_ps):
                    for cv in range(CT):
                        nc.tensor.matmul(u_ps[cv],
                                         vt_sb[:, ktp, :, cv * P:(cv + 1) * P],
                                         ep, perf_mode=DR,
                                         start=(ktp == 0), stop=(ktp == KTP - 1))
                    nc.tensor.matmul(r_ps, onesp_sb, ep, perf_mode=DR,
                                     start=(ktp == 0), stop=(ktp == KTP - 1))

                for ktp in range(KTP):
                    ep = pe.tile([P, 2, FD], FP8, tag="e", name=f"e{qc}_{ktp}")
                    for i in range(2):
                        kt = 2 * ktp + i
                        sps = ps_s.tile([P, FD], F32, tag="s", name=f"s{qc}_{kt}")
                        for ctp in range(CT // 2):
                            nc.tensor.matmul(sps,
                                             k_sb[:, ctp, :, kt * P:(kt + 1) * P],
                                             q_sb[:, ctp, :, qcols],
                                             perf_mode=DR,
                                             start=(ctp == 0),
                                             stop=(ctp == CT // 2 - 1))
                        nc.scalar.activation(out=ep[:, i, :], in_=sps, func=AF.Exp)
                    pend.append((ep, ktp))
                    if len(pend) > 2:
                        emit_u(*pend.pop(0))
                for item in pend:
                    emit_u(*item)

                rr = psm.tile([P, FD], F32, tag="rr")
                nc.vector.reciprocal(out=rr, in_=r_ps)
                for cv in range(CT):
                    nc.vector.tensor_tensor(attn_sb[:, cv, qcols], u_ps[cv], rr,
                                            ALU.mult)

                # output projection + bias + residual for this q-chunk;
                # qc=0's projection overlaps qc=1's attention on the PE.
                proj_pool = ps_m if qc == 0 else ps_s
                for ot in range(CT):
                    ops = proj_pool.tile([P, FD], F32,
                                         tag="m" if qc == 0 else "s",
                                         name=f"proj{qc}_{ot}")
                    for cv in range(CT):
                        nc.tensor.matmul(ops,
                                         w_sb["wo"][:, cv, ot * P:(ot + 1) * P],
                                         attn_sb[:, cv, qcols],
                                         start=(cv == 0), stop=(cv == CT - 1))
                    o_sb = po.tile([P, FD], F32, tag="o", name=f"o{qc}_{ot}")
                    nc.vector.tensor_tensor(o_sb, ops, x_sb[:, ot, qcols], ALU.add)
                    nc.vector.tensor_scalar(out=o_sb, in0=o_sb,
                                            scalar1=boe_sb[:, ot:ot + 1],
                                            scalar2=None, op0=ALU.add)
                    nc.sync.dma_start(out=out_d[ot * P:(ot + 1) * P, qcols],
                                      in_=o_sb)
    nc.compile()
    return nc


def make_core_inputs(x, gn_w, gn_b, wq, bq, wk, bk, wv, bv, wo, bo):
    """Build the 8 per-core input maps from full inputs."""
    bf16 = ml_dtypes.bfloat16
    f32 = np.float32
    b = x.shape[0]
    xf = np.ascontiguousarray(np.asarray(x, f32).reshape(b, C, NK))

    def wslice(w):
        wT = np.ascontiguousarray(np.asarray(w, f32).T.astype(bf16))
        return np.ascontiguousarray(wT.reshape(CT, P, C))

    # fold the attention scale into the Q projection weights + bias
    wqT = wslice(np.asarray(wq, f32) * SCALE)
    wkT, wvT, woT = wslice(wk), wslice(wv), wslice(wo)

    def percol(v):  # (512,) -> (128, 4): [p, ct]
        return np.ascontiguousarray(np.asarray(v, f32).reshape(CT, P).T)

    bqs = percol(np.asarray(bq, f32) * SCALE)
    bo_eff = percol(np.asarray(bo, np.float64)
                    + np.asarray(wo, np.float64) @ np.asarray(bv, np.float64))
    gam = percol(gn_w)
    bet = percol(gn_b)

    indf = np.zeros((P, CT, G), f32)
    indb = np.zeros((P, CT, P), f32)
    for ct in range(CT):
        for p in range(P):
            g = (ct * P + p) // GS
            indf[p, ct, g] = 1.0 / GS
            indb[g, ct, p] = 1.0
    shared = dict(wqT=wqT, wkT=wkT, wvT=wvT, woT=woT, bqs=bqs, boe=bo_eff,
                  gam=gam, bet=bet, indf=indf, indb=indb)

    in_maps = []
    for core in range(N_CORES):
        bb, qb = core // 4, core % 4
        qs = qb * NQ
        xr = np.ascontiguousarray(
            np.concatenate([xf[bb][:, qs:], xf[bb][:, :qs]], axis=1))
        in_maps.append(dict(x=xr, **shared))
    return in_maps


_NC_CACHE = None


def _get_nc():
    global _NC_CACHE
    if _NC_CACHE is None:
        _NC_CACHE = build_bass()
    return _NC_CACHE


def run_on_cores(in_maps, **kw):
    from concourse.bass_utils import run_bass_kernel_spmd
    nc = _get_nc()
    return run_bass_kernel_spmd(nc, in_maps, core_ids=list(range(N_CORES)), **kw)


def kernel(**inputs):
    x = np.asarray(inputs["x"])
    b, c, H, W = x.shape
    in_maps = make_core_inputs(**inputs)
    res = run_on_cores(in_maps)
    out = np.zeros((b, C, NK), np.float32)
    for core in range(N_CORES):
        bb, qb = core // 4, core % 4
        out[bb][:, qb * NQ:(qb + 1) * NQ] = res.results[core]["out"]
    return out.reshape(b, c, H, W)


# revision 17
# speedup vs baseline: 1.3558x; 1.1713x over previous
"""Trainium2 Bass kernel for nn_AttentionBlock (GroupNorm -> QKV 1x1 -> spatial
self-attention -> out-proj + residual), sharded over 8 NeuronCores.

Sharding: data-parallel over batch (2) x query-block (4). Each core gets its
batch image with pixel columns rolled so its 1024 queries are columns 0:1024
(attention + GroupNorm are permutation-invariant over key pixels), computes
K/V over all 4096 keys, and emits its (512, 1024) output slice.

Numerics: all matmuls in bf16 with f32 PSUM accumulation; GroupNorm stats,
softmax normalization and residual in f32.  bk is dropped (additive per-query
score constant, softmax-invariant); bv is folded into the output-proj bias
(softmax rows sum to 1), so bo_eff = bo + wo @ bv.
"""

import numpy as np
import ml_dtypes

import concourse.bass as bass
import concourse.bacc as bacc
import concourse.mybir as mybir
import concourse.tile as tile

F32 = mybir.dt.float32
BF16 = mybir.dt.bfloat16
FP8 = mybir.dt.float8e4
DR = mybir.MatmulPerfMode.DoubleRow
AF = mybir.ActivationFunctionType
ALU = mybir.AluOpType

P = 128
C = 512          # channels
CT = C // P      # 4 channel tiles
NK = 4096        # key pixels per batch image
KT = NK // P     # 32 key tiles
NQ = 1024        # queries per core
FD = 512         # matmul free-dim chunk
NCH = NK // FD   # 8 column chunks
G = 32           # groups
GS = C // G      # 16 channels per group
EPS = 1e-5
SCALE = float(C) ** -0.5
N_CORES = 8


def build_bass():
    nc = bacc.Bacc("TRN2", target_bir_lowering=False, debug=False,
                   num_devices=N_CORES)

    x_d = nc.dram_tensor("x", (C, NK), F32, kind="ExternalInput").ap()
    wq_d = nc.dram_tensor("wqT", (CT, P, C), BF16, kind="ExternalInput").ap()
    wk_d = nc.dram_tensor("wkT", (P, CT // 2, 2, C), FP8, kind="ExternalInput").ap()
    wv_d = nc.dram_tensor("wvT", (P, CT // 2, 2, C), FP8, kind="ExternalInput").ap()
    wo_d = nc.dram_tensor("woT", (CT, P, C), BF16, kind="ExternalInput").ap()
    bqs_d = nc.dram_tensor("bqs", (P, CT), F32, kind="ExternalInput").ap()
    boe_d = nc.dram_tensor("boe", (P, CT), F32, kind="ExternalInput").ap()
    gam_d = nc.dram_tensor("gam", (P, CT), F32, kind="ExternalInput").ap()
    bet_d = nc.dram_tensor("bet", (P, CT), F32, kind="ExternalInput").ap()
    indf_d = nc.dram_tensor("indf", (P, CT, G), F32, kind="ExternalInput").ap()
    indb_d = nc.dram_tensor("indb", (P, CT, P), F32, kind="ExternalInput").ap()
    out_d = nc.dram_tensor("out", (C, NQ), F32, kind="ExternalOutput").ap()

    with tile.TileContext(nc) as tc:
        with (
            tc.tile_pool(name="px", bufs=1) as px,
            tc.tile_pool(name="pw", bufs=1) as pw,
            tc.tile_pool(name="pc", bufs=1) as pcst,
            tc.tile_pool(name="ph", bufs=3) as ph,
            tc.tile_pool(name="pkvq", bufs=1) as pkvq,
            tc.tile_pool(name="pe", bufs=4) as pe,
            tc.tile_pool(name="psm", bufs=2) as psm,
            tc.tile_pool(name="po", bufs=2) as po,
            tc.tile_pool(name="ps_u", bufs=4, space="PSUM") as ps_u,
            tc.tile_pool(name="ps_r", bufs=1, space="PSUM") as ps_r,
            tc.tile_pool(name="ps_s", bufs=2, space="PSUM") as ps_s,
            tc.tile_pool(name="ps_m", bufs=1, space="PSUM") as ps_m,
        ):
            # ---- PE warmup: dummy matmul chain spanning the x-DMA prologue
            # keeps the HAM clock-gate at K=8/8 so the first real matmuls
            # run at 2.4 GHz instead of 1.2 GHz.
            ones_sb = pcst.tile([P, P], BF16, tag="ones")
            nc.vector.memset(ones_sb, 1.0)
            warm_rhs = pcst.tile([P, FD], BF16, tag="wrm")
            nc.vector.memset(warm_rhs, 0.0)
            wsink = pcst.tile([P, 1], F32, tag="wsink")
            NWARM = 160
            wps = ps_m.tile([P, FD], F32, tag="m", name="warm")
            for i in range(NWARM):
                nc.tensor.matmul(wps, ones_sb, warm_rhs,
                                 start=(i == 0), stop=(i == NWARM - 1))
            nc.vector.tensor_copy(out=wsink, in_=wps[:, 0:1])

            # ---- x load (first on the DMA queues) + GroupNorm statistics ----
            x_sb = px.tile([P, CT, NK], F32, tag="x")
            stats = pcst.tile([P, CT, NCH, 6], F32, tag="stats")
            mv = pcst.tile([P, CT, 2], F32, tag="mv")
            cstats = pcst.tile([P, CT, 2], F32, tag="cstats")
            XDH = 2048  # DMA chunk columns (1 MB) so bn_stats trails the DMA
            for ct in range(CT):
                for xc in range(NK // XDH):
                    xcols = slice(xc * XDH, (xc + 1) * XDH)
                    nc.sync.dma_start(out=x_sb[:, ct, xcols],
                                      in_=x_d[ct * P:(ct + 1) * P, xcols])
                    for s in range(xc * (XDH // FD), (xc + 1) * (XDH // FD)):
                        nc.vector.bn_stats(out=stats[:, ct, s, :],
                                           in_=x_sb[:, ct, s * FD:(s + 1) * FD])
                nc.vector.bn_aggr(out=mv[:, ct, :], in_=stats[:, ct])
                # cstats = [mean_c, var_c + mean_c^2]
                nc.scalar.activation(out=cstats[:, ct, 1:2], in_=mv[:, ct, 0:1],
                                     func=AF.Square)
                nc.vector.tensor_tensor(cstats[:, ct, 1:2], cstats[:, ct, 1:2],
                                        mv[:, ct, 1:2], ALU.add)
                nc.vector.tensor_copy(out=cstats[:, ct, 0:1], in_=mv[:, ct, 0:1])

            # ---- constant / weight loads ----
            w_sb = {}
            for nm, d in (("wq", wq_d), ("wo", wo_d)):
                t = pw.tile([P, CT, C], BF16, tag=nm)
                for ct in range(CT):
                    nc.sync.dma_start(out=t[:, ct, :], in_=d[ct])
                w_sb[nm] = t
            for nm, d in (("wk", wk_d), ("wv", wv_d)):
                t = pw.tile([P, CT // 2, 2, C], FP8, tag=nm)
                nc.sync.dma_start(out=t, in_=d)
                w_sb[nm] = t
            bqs_sb = pcst.tile([P, CT], F32, tag="bqs")
            nc.sync.dma_start(out=bqs_sb, in_=bqs_d)
            boe_sb = pcst.tile([P, CT], F32, tag="boe")
            nc.sync.dma_start(out=boe_sb, in_=boe_d)
            gam_sb = pcst.tile([P, CT], F32, tag="gam")
            nc.sync.dma_start(out=gam_sb, in_=gam_d)
            bet_sb = pcst.tile([P, CT], F32, tag="bet")
            nc.sync.dma_start(out=bet_sb, in_=bet_d)
            indf_sb = pcst.tile([P, CT, G], F32, tag="indf")
            nc.sync.dma_start(out=indf_sb, in_=indf_d)
            indb_sb = pcst.tile([P, CT, P], F32, tag="indb")
            nc.sync.dma_start(out=indb_sb, in_=indb_d)
            eps_sb = pcst.tile([P, 1], F32, tag="eps")
            nc.vector.memset(eps_sb, EPS)

            # group combine: [32, 2] = sum_ct indf^T @ cstats  (weights 1/16)
            gps = ps_m.tile([G, 2], F32, tag="m")
            for ct in range(CT):
                nc.tensor.matmul(gps, indf_sb[:, ct, :], cstats[:, ct, :],
                                 start=(ct == 0), stop=(ct == CT - 1))
            gsb = pcst.tile([P, 2], F32, tag="gsb")
            nc.vector.tensor_copy(out=gsb[0:G, :], in_=gps)
            # grhs = [mu_g, rstd_g], zero-padded to 128 partitions
            grhs = pcst.tile([P, 2], F32, tag="grhs")
            nc.vector.memset(grhs, 0.0)
            sq = pcst.tile([P, 1], F32, tag="sq")
            nc.scalar.activation(out=sq[0:G], in_=gsb[0:G, 0:1], func=AF.Square)
            nc.vector.tensor_tensor(sq[0:G], gsb[0:G, 1:2], sq[0:G], ALU.subtract)
            nc.scalar.activation(out=sq[0:G], in_=sq[0:G], func=AF.Sqrt,
                                 bias=eps_sb[0:G])
            nc.vector.tensor_copy(out=grhs[0:G, 0:1], in_=gsb[0:G, 0:1])
            nc.vector.reciprocal(out=grhs[0:G, 1:2], in_=sq[0:G])

            # broadcast to per-channel scale/shift: h = x*A + B
            A_sb = pcst.tile([P, CT], F32, tag="A")
            B_sb = pcst.tile([P, CT], F32, tag="B")
            for ct in range(CT):
                abps = ps_m.tile([P, 2], F32, tag="m")
                nc.tensor.matmul(abps, indb_sb[:, ct, :], grhs, start=True, stop=True)
                nc.vector.tensor_tensor(A_sb[:, ct:ct + 1], abps[:, 1:2],
                                        gam_sb[:, ct:ct + 1], ALU.mult)
                nc.vector.tensor_tensor(B_sb[:, ct:ct + 1], abps[:, 0:1],
                                        A_sb[:, ct:ct + 1], ALU.mult)
                nc.vector.tensor_tensor(B_sb[:, ct:ct + 1], bet_sb[:, ct:ct + 1],
                                        B_sb[:, ct:ct + 1], ALU.subtract)

            # ---- GN apply + Q/K/Vt projections, per 512-column chunk ----
            # K/Q/Vt stored as fp8 e4m3 in DoubleRow pair layout [P, pair, 2, n]
            k_sb = pkvq.tile([P, CT // 2, 2, NK], FP8, tag="K")
            vt_sb = pkvq.tile([P, KT // 2, 2, FD], FP8, tag="Vt")
            q_sb = pkvq.tile([P, CT // 2, 2, NQ], FP8, tag="Q")
            onesp_sb = pcst.tile([P, 2, P], FP8, tag="onesp")
            nc.vector.memset(onesp_sb, 1.0)
            for ch in range(NCH):
                cols = slice(ch * FD, (ch + 1) * FD)
                h_ch = ph.tile([P, CT // 2, 2, FD], FP8, tag="h")
                for ct in range(CT):
                    nc.scalar.activation(
                        out=h_ch[:, ct // 2, ct % 2, :], in_=x_sb[:, ct, cols],
                        func=AF.Identity, bias=B_sb[:, ct:ct + 1],
                        scale=A_sb[:, ct:ct + 1])
                if ch < NQ // FD:
                    hq_ch = ph.tile([P, CT, FD], BF16, tag="hq")
                    for ct in range(CT):
                        nc.scalar.activation(
                            out=hq_ch[:, ct, :], in_=x_sb[:, ct, cols],
                            func=AF.Identity, bias=B_sb[:, ct:ct + 1],
                            scale=A_sb[:, ct:ct + 1])
                # K chunk: K[ot, cols] = sum_ct wkT[ct][:,ot]^T @ h[ct, cols]
                for ot in range(CT):
                    kps = ps_u.tile([P, FD], F32, tag="u")
                    for ctp in range(CT // 2):
                        nc.tensor.matmul(kps,
                                         w_sb["wk"][:, ctp, :, ot * P:(ot + 1) * P],
                                         h_ch[:, ctp, :, :], perf_mode=DR,
                                         start=(ctp == 0),
                                         stop=(ctp == CT // 2 - 1))
                    nc.vector.tensor_copy(out=k_sb[:, ot // 2, ot % 2, cols],
                                          in_=kps)
                # Vt chunk: Vt[kt] = sum_ct h[ct, kt]^T @ wvT[ct]
                for kk in range(FD // P):
                    kt = ch * (FD // P) + kk
                    vps = ps_u.tile([P, FD], F32, tag="u")
                    for ctp in range(CT // 2):
                        nc.tensor.matmul(vps,
                                         h_ch[:, ctp, :, kk * P:(kk + 1) * P],
                                         w_sb["wv"][:, ctp, :, :], perf_mode=DR,
                                         start=(ctp == 0),
                                         stop=(ctp == CT // 2 - 1))
                    nc.vector.tensor_copy(out=vt_sb[:, kt // 2, kt % 2, :],
                                          in_=vps)
                # Q chunk (first 1024 columns only), scaled by C^-0.5, + bq
                if ch < NQ // FD:
                    for ot in range(CT):
                        qps = ps_u.tile([P, FD], F32, tag="u")
                        for ct in range(CT):
                            nc.tensor.matmul(qps,
                                             w_sb["wq"][:, ct, ot * P:(ot + 1) * P],
                                             hq_ch[:, ct, :],
                                             start=(ct == 0), stop=(ct == CT - 1))
                        # SCALE is folded into wqT and bqs host-side
                        nc.scalar.activation(out=q_sb[:, ot // 2, ot % 2, cols],
                                             in_=qps, func=AF.Identity,
                                             bias=bqs_sb[:, ot:ot + 1])

            # ---- attention: St = K^T Q per k-tile, exp, U += Vt^T E, r += 1^T E
            attn_sb = pkvq.tile([P, CT, NQ], BF16, tag="attn")
            for qc in range(NQ // FD):
                qcols = slice(qc * FD, (qc + 1) * FD)
                u_ps = [ps_u.tile([P, FD], F32, tag="u", name=f"u{qc}_{cv}")
                        for cv in range(CT)]
                r_ps = ps_r.tile([P, FD], F32, tag="r")
                KTP = KT // 2
                pend = []

                def emit_u(ep, ktp, qc=qc, u_ps=u_ps, r_ps=r_ps):
                    for cv in range(CT):
                        nc.tensor.matmul(u_ps[cv],
                                         vt_sb[:, ktp, :, cv * P:(cv + 1) * P],
                                         ep, perf_mode=DR,
                                         start=(ktp == 0), stop=(ktp == KTP - 1))
                    nc.tensor.matmul(r_ps, onesp_sb, ep, perf_mode=DR,
                                     start=(ktp == 0), stop=(ktp == KTP - 1))

                for ktp in range(KTP):
                    ep = pe.tile([P, 2, FD], FP8, tag="e", name=f"e{qc}_{ktp}")
                    for i in range(2):
                        kt = 2 * ktp + i
                        sps = ps_s.tile([P, FD], F32, tag="s", name=f"s{qc}_{kt}")
                        for ctp in range(CT // 2):
                            nc.tensor.matmul(sps,
                                             k_sb[:, ctp, :, kt * P:(kt + 1) * P],
                                             q_sb[:, ctp, :, qcols],
                                             perf_mode=DR,
                                             start=(ctp == 0),
                                             stop=(ctp == CT // 2 - 1))
                        nc.scalar.activation(out=ep[:, i, :], in_=sps, func=AF.Exp)
                    pend.append((ep, ktp))
                    if len(pend) > 2:
                        emit_u(*pend.pop(0))
                for item in pend:
                    emit_u(*item)

                rr = psm.tile([P, FD], F32, tag="rr")
                nc.vector.reciprocal(out=rr, in_=r_ps)
                for cv in range(CT):
                    nc.vector.tensor_tensor(attn_sb[:, cv, qcols], u_ps[cv], rr,
                                            ALU.mult)

                # output projection + bias + residual for this q-chunk;
                # qc=0's projection overlaps qc=1's attention on the PE.
                proj_pool = ps_m if qc == 0 else ps_s
                for ot in range(CT):
                    ops = proj_pool.tile([P, FD], F32,
                                         tag="m" if qc == 0 else "s",
                                         name=f"proj{qc}_{ot}")
                    for cv in range(CT):
                        nc.tensor.matmul(ops,
                                         w_sb["wo"][:, cv, ot * P:(ot + 1) * P],
                                         attn_sb[:, cv, qcols],
                                         start=(cv == 0), stop=(cv == CT - 1))
                    o_sb = po.tile([P, FD], F32, tag="o", name=f"o{qc}_{ot}")
                    nc.scalar.activation(out=o_sb, in_=ops, func=AF.Identity,
                                         bias=boe_sb[:, ot:ot + 1])
                    nc.vector.tensor_tensor(o_sb, o_sb, x_sb[:, ot, qcols],
                                            ALU.add)
                    nc.sync.dma_start(out=out_d[ot * P:(ot + 1) * P, qcols],
                                      in_=o_sb)
    nc.compile()
    return nc


def make_core_inputs(x, gn_w, gn_b, wq, bq, wk, bk, wv, bv, wo, bo):
    """Build the 8 per-core input maps from full inputs."""
    bf16 = ml_dtypes.bfloat16
    f32 = np.float32
    b = x.shape[0]
    xf = np.ascontiguousarray(np.asarray(x, f32).reshape(b, C, NK))

    def wslice(w):
        wT = np.ascontiguousarray(np.asarray(w, f32).T.astype(bf16))
        return np.ascontiguousarray(wT.reshape(CT, P, C))

    # fold the attention scale into the Q projection weights + bias
    wqT = wslice(np.asarray(wq, f32) * SCALE)
    woT = wslice(wo)

    def wpair(w):  # (512,512) w[o,c] -> fp8 pair layout [p, ctp, i, o]
        wT = np.asarray(w, f32).T.astype(ml_dtypes.float8_e4m3)
        return np.ascontiguousarray(
            wT.reshape(CT // 2, 2, P, C).transpose(2, 0, 1, 3))

    wkT, wvT = wpair(wk), wpair(wv)

    def percol(v):  # (512,) -> (128, 4): [p, ct]
        return np.ascontiguousarray(np.asarray(v, f32).reshape(CT, P).T)

    bqs = percol(np.asarray(bq, f32) * SCALE)
    bo_eff = percol(np.asarray(bo, np.float64)
                    + np.asarray(wo, np.float64) @ np.asarray(bv, np.float64))
    gam = percol(gn_w)
    bet = percol(gn_b)

    indf = np.zeros((P, CT, G), f32)
    indb = np.zeros((P, CT, P), f32)
    for ct in range(CT):
        for p in range(P):
            g = (ct * P + p) // GS
            indf[p, ct, g] = 1.0 / GS
            indb[g, ct, p] = 1.0
    shared = dict(wqT=wqT, wkT=wkT, wvT=wvT, woT=woT, bqs=bqs, boe=bo_eff,
                  gam=gam, bet=bet, indf=indf, indb=indb)

    in_maps = []
    for core in range(N_CORES):
        bb, qb = core // 4, core % 4
        qs = qb * NQ
        xr = np.ascontiguousarray(
            np.concatenate([xf[bb][:, qs:], xf[bb][:, :qs]], axis=1))
        in_maps.append(dict(x=xr, **shared))
    return in_maps


_NC_CACHE = None


def _get_nc():
    global _NC_CACHE
    if _NC_CACHE is None:
        _NC_CACHE = build_bass()
    return _NC_CACHE


def run_on_cores(in_maps, **kw):
    from concourse.bass_utils import run_bass_kernel_spmd
    nc = _get_nc()
    return run_bass_kernel_spmd(nc, in_maps, core_ids=list(range(N_CORES)), **kw)


def kernel(**inputs):
    x = np.asarray(inputs["x"])
    b, c, H, W = x.shape
    in_maps = make_core_inputs(**inputs)
    res = run_on_cores(in_maps)
    out = np.zeros((b, C, NK), np.float32)
    for core in range(N_CORES):
        bb, qb = core // 4, core % 4
        out[bb][:, qb * NQ:(qb + 1) * NQ] = res.results[core]["out"]
    return out.reshape(b, c, H, W)
